# revision 11
# baseline (speedup 1.0000x reference)
"""CosineAttention on 8 TRN2 NeuronCores.

Sharding: head-parallel attention + AllToAll shard-transpose +
token-parallel out-projection.

  core c owns head h=c for both batches:
    - stage 1 (per 512-token chunk): [q;k]T and vT via weight-stationary
      bf16 matmuls over xT; vT DMA-XBAR-transposed (ACT queue) into packed
      [j, 64] tiles -> [j, 65] vo tiles (ones col 64); k remapped to
      partitions 0-63; per-j k sum-of-squares via ones-matmul into PSUM;
      per-token q sum-of-squares via GpSimd partition-reduce.
    - per-batch norm: rq = 1/(sqrt(mean q^2)+eps) batched over 2048
      tokens; rk = 1/(sqrt(sum k^2)+8eps) per j (folds SCALE=1/8);
      qn = q * partition_broadcast(rq).
    - phase 2 (per 512-token i-chunk): simT[j,i] = kraw^T qn; exp on ACT
      with per-partition scale AP rk[j]; attn@[v|1] accumulates so row 64
      is the softmax denominator Z; oc = av[0:64] * partition_broadcast
      (1/Z) in bf16.  Stage-1 chunks of batch 1 interleave with batch-0
      attention to keep all engines busy.
    - ONE AllToAll [512, 512] bf16: shard g = [64, 512] feature tile for
      token block g -> core receives all 512 features for its own 512
      tokens.
    - token-parallel out-proj with the full Wout (16 matmuls) -> outT
      [512 features, 512 tokens] f32; host concatenates token blocks.
"""

import numpy as np
import ml_dtypes

import concourse.bass as bass
import concourse.tile as tile
from concourse import bacc
import concourse.mybir as mybir
from concourse import bass_utils

f32 = mybir.dt.float32
f32r = mybir.dt.float32r
bf16 = mybir.dt.bfloat16
AF = mybir.ActivationFunctionType
ALU = mybir.AluOpType

N_CORES = 8
HEADS = 8
D = 64            # head dim
B = 2             # batch
SEQ = 2048        # tokens per batch
DIM = 512         # model dim
NTOK = B * SEQ    # 4096
EPS = 1e-4
SCALE = D ** -0.5  # 0.125

CH = 512          # token chunk = A2A shard = out-proj block
NCH = NTOK // CH  # 8
JPC = CH // 128   # 4 j-tiles per chunk
JPB = SEQ // 128  # 16 j-tiles per batch

_BUILD_CACHE = {}


def build(num_devices=N_CORES, collective=True):
    key = (num_devices, collective)
    if key in _BUILD_CACHE:
        return _BUILD_CACHE[key]
    nc = bacc.Bacc("TRN2", target_bir_lowering=False, debug=False,
                   num_devices=num_devices)
    xT = nc.dram_tensor("xT", [DIM, NTOK], bf16, kind="ExternalInput").ap()
    wqk = nc.dram_tensor("wqk", [DIM, 128], bf16, kind="ExternalInput").ap()
    wv = nc.dram_tensor("wv", [DIM, D], bf16, kind="ExternalInput").ap()
    w2 = nc.dram_tensor("w2", [DIM, DIM], bf16, kind="ExternalInput").ap()
    o64 = nc.dram_tensor("o64", [D, 1], bf16, kind="ExternalInput").ap()
    outT = nc.dram_tensor("outT", [DIM, CH], f32, kind="ExternalOutput").ap()

    xTr = xT.rearrange("(t p) n -> p t n", p=128)
    w2r = w2.rearrange("(t p) n -> p t n", p=128)
    wqkr = wqk.rearrange("(t p) m -> p t m", p=128)
    wvr = wv.rearrange("(t p) m -> p t m", p=128)
    outTr = outT.rearrange("(mt p) n -> p mt n", p=128)

    with tile.TileContext(nc) as tc:
        with (
            tc.tile_pool(name="persist", bufs=1) as pp,
            tc.tile_pool(name="sb", bufs=2) as sb,
            tc.tile_pool(name="ps", bufs=1, space="PSUM") as ps,
            tc.tile_pool(name="dram", bufs=1, space="DRAM") as dram,
            nc.allow_low_precision(reason="bf16 matmul path"),
        ):
            # ---- persistent weights / constants ----
            wqk_sb = pp.tile([128, 4, 128], bf16)
            wv_sb = pp.tile([128, 4, D], bf16)
            w2_sb = pp.tile([128, 4, DIM], bf16)
            nc.sync.dma_start(wqk_sb[:], wqkr[:])
            nc.sync.dma_start(wv_sb[:], wvr[:])
            nc.sync.dma_start(w2_sb[:], w2r[:])
            o64_sb = pp.tile([D, 1], bf16)
            nc.sync.dma_start(o64_sb[:], o64[:])

            # ---- persistent activations ----
            qk_all = pp.tile([128, NTOK], bf16)   # [qT; kT] raw
            qn_sb = pp.tile([D, NTOK], bf16)      # normalized qT
            kraw_sb = pp.tile([D, NTOK], bf16)    # raw kT at partitions 0-63
            vo_sb = pp.tile([128, NCH * JPC, D + 1], bf16)  # [v | ones]
            rtq_sb = pp.tile([1, NTOK], f32)      # sqrt(mean q^2) per token
            rks_sb = pp.tile([128, NCH * JPC], f32)  # 1/(|k|+8eps) per j
            nc.gpsimd.memset(vo_sb[:, :, D:D + 1], 1.0)

            cc_in = dram.tile([DIM, CH], bf16, name="cc_in")
            cc_out = dram.tile([DIM, CH], bf16, name="cc_out")

            stk_ps = ps.tile([128, NCH * JPC], f32, tag="stk", bufs=1)

            def stage1_chunk(ci):
                cols = slice(ci * CH, (ci + 1) * CH)
                xt = sb.tile([128, 4, CH], bf16, tag="xt")
                nc.sync.dma_start(xt[:], xTr[:, :, cols])
                qk_ps = ps.tile([128, CH], f32, tag="a", bufs=4)
                for t in range(4):
                    nc.tensor.matmul(qk_ps[:], wqk_sb[:, t, :], xt[:, t, :],
                                     start=(t == 0), stop=(t == 3))
                vt_ps = ps.tile([D, CH], f32, tag="b", bufs=2)
                for t in range(4):
                    nc.tensor.matmul(vt_ps[:], wv_sb[:, t, :], xt[:, t, :],
                                     start=(t == 0), stop=(t == 3))
                nc.vector.tensor_copy(qk_all[:, cols], qk_ps[:])
                # raw kT at partitions 0-63 (DMA partition remap, ACT queue)
                nc.scalar.dma_start(kraw_sb[:, cols], qk_all[64:128, cols])
                # vT -> packed [j, d] tiles via DMA XBAR transpose (ACT queue)
                vt_sb = sb.tile([D, CH], bf16, tag="vtsb")
                nc.vector.tensor_copy(vt_sb[:], vt_ps[:])
                for jj in range(JPC):
                    jt = ci * JPC + jj
                    js = slice(jj * 128, (jj + 1) * 128)
                    vtr = sb.tile([128, D], bf16, tag="vtr", bufs=4)
                    nc.scalar.dma_start_transpose(vtr[:], vt_sb[:, js])
                    nc.gpsimd.tensor_copy(vo_sb[:, jt, 0:D], vtr[:])
                # k sum of squares per j (ones-matmul on remapped kraw)
                ksq = sb.tile([D, CH], bf16, tag="ksq")
                nc.vector.tensor_mul(ksq[:], kraw_sb[:, cols], kraw_sb[:, cols])
                for jj in range(JPC):
                    jt = ci * JPC + jj
                    js = slice(jj * 128, (jj + 1) * 128)
                    nc.tensor.matmul(stk_ps[:, jt:jt + 1], ksq[:, js],
                                     o64_sb[:], start=True, stop=True)
                # q sum of squares per token (GpSimd partition reduce)
                sq_q = sb.tile([D, CH], bf16, tag="sqq")
                nc.vector.tensor_mul(sq_q[:], qk_all[0:D, cols],
                                     qk_all[0:D, cols])
                stq_sb = sb.tile([1, CH], f32, tag="stq")
                nc.gpsimd.tensor_reduce(stq_sb[:], sq_q[:],
                                        mybir.AxisListType.C, ALU.add)
                nc.scalar.activation(rtq_sb[:, cols], stq_sb[:], AF.Sqrt,
                                     scale=1.0 / D)

            def norm_half(h):
                # k reciprocal for batch h
                hs = slice(h * JPB, (h + 1) * JPB)
                skh = sb.tile([128, JPB], f32, tag="skh")
                nc.scalar.activation(skh[:], stk_ps[:, hs], AF.Sqrt)
                seh = sb.tile([128, JPB], f32, tag="seh")
                nc.vector.tensor_scalar_add(seh[:], skh[:], 8.0 * EPS)
                nc.vector.reciprocal(rks_sb[:, hs], seh[:])
                # q normalization for batch h, batched over 2048 tokens
                tok = slice(h * SEQ, (h + 1) * SEQ)
                req = sb.tile([1, SEQ], f32, tag="req")
                nc.vector.tensor_scalar_add(req[:], rtq_sb[:, tok], EPS)
                rcq = sb.tile([1, SEQ], f32, tag="rcq")
                nc.vector.reciprocal(rcq[:], req[:])
                rcq_b = sb.tile([1, SEQ], bf16, tag="rcqb")
                nc.vector.tensor_copy(rcq_b[:], rcq[:])
                for cj in range(NCH // B):
                    cols = slice(h * SEQ + cj * CH, h * SEQ + (cj + 1) * CH)
                    rb_sb = sb.tile([D, CH], bf16, tag="rbsb")
                    nc.gpsimd.partition_broadcast(
                        rb_sb[:], rcq_b[:, cj * CH:(cj + 1) * CH])
                    nc.vector.tensor_mul(qn_sb[:, cols], qk_all[0:D, cols],
                                         rb_sb[:])

            def phase2_chunk(b, ch):
                g = b * (NCH // B) + ch
                i0 = g * CH
                expT = sb.tile([128, JPB, CH], bf16, tag="exp")
                for jt in range(JPB):
                    j0 = b * SEQ + jt * 128
                    sim_ps = ps.tile([128, CH], f32, tag="a", bufs=4)
                    nc.tensor.matmul(sim_ps[:], kraw_sb[:, j0:j0 + 128],
                                     qn_sb[:, i0:i0 + CH],
                                     start=True, stop=True)
                    gj = b * JPB + jt
                    nc.scalar.activation(expT[:, jt, :], sim_ps[:], AF.Exp,
                                         scale=rks_sb[:, gj:gj + 1])
                av_ps = ps.tile([D + 1, CH], f32, tag="b", bufs=2)
                for jt in range(JPB):
                    nc.tensor.matmul(av_ps[:], vo_sb[:, b * JPB + jt, :],
                                     expT[:, jt, :],
                                     start=(jt == 0), stop=(jt == JPB - 1))
                rse = sb.tile([1, CH], f32, tag="rse")
                nc.vector.reciprocal(rse[:], av_ps[D:D + 1, :])
                r2_sb = sb.tile([D, CH], f32, tag="r2sb")
                nc.gpsimd.partition_broadcast(r2_sb[:], rse[:])
                oc = sb.tile([D, CH], bf16, tag="oc")
                nc.vector.tensor_mul(oc[:], av_ps[0:D, :], r2_sb[:])
                nc.sync.dma_start(cc_in[g * D:(g + 1) * D, :], oc[:])

            # ---- schedule ----
            for ci in range(4):
                stage1_chunk(ci)
            norm_half(0)
            for ch in range(4):
                stage1_chunk(4 + ch)
                phase2_chunk(0, ch)
            norm_half(1)
            for ch in range(4):
                phase2_chunk(1, ch)

            # ---- shard transpose: one AllToAll ----
            if collective:
                nc.gpsimd.collective_compute(
                    "AllToAll", ALU.bypass,
                    replica_groups=[list(range(num_devices))],
                    ins=[cc_in[:]], outs=[cc_out[:]])
            else:
                # timing-only stand-in (numerically wrong off-diagonal)
                nc.sync.dma_start(cc_out[:], cc_in[:])

            # ---- token-parallel out-projection ----
            ag = sb.tile([128, 4, CH], bf16, tag="ag")
            nc.sync.dma_start(ag[:], cc_out.rearrange("(t p) n -> p t n",
                                                      p=128))
            fo = sb.tile([128, 4, CH], f32, tag="fo")
            for mt in range(4):
                fp_ps = ps.tile([128, CH], f32, tag="a", bufs=4)
                for t in range(4):
                    nc.tensor.matmul(fp_ps[:],
                                     w2_sb[:, t, mt * 128:(mt + 1) * 128],
                                     ag[:, t, :], start=(t == 0), stop=(t == 3))
                nc.vector.tensor_copy(fo[:, mt, :], fp_ps[:])
            nc.sync.dma_start(outTr[:], fo[:])
    nc.compile()
    _BUILD_CACHE[key] = nc
    return nc


def make_in_maps(x, Wq, Wkv, Wout):
    xT = np.ascontiguousarray(
        x.reshape(NTOK, DIM).T).astype(ml_dtypes.bfloat16)
    w2 = np.ascontiguousarray(Wout.T).astype(ml_dtypes.bfloat16)
    o64 = np.ones((D, 1), ml_dtypes.bfloat16)
    in_maps = []
    for c in range(N_CORES):
        rows = slice(c * D, (c + 1) * D)
        wqk = np.ascontiguousarray(
            np.concatenate([Wq[rows, :].T, Wkv[rows, :].T],
                           axis=1)).astype(ml_dtypes.bfloat16)
        wv = np.ascontiguousarray(
            Wkv[DIM + c * D:DIM + (c + 1) * D, :].T).astype(ml_dtypes.bfloat16)
        in_maps.append({
            "xT": xT, "wqk": wqk, "wv": wv, "w2": w2, "o64": o64,
        })
    return in_maps


def kernel(x, Wq, Wkv, Wout, _trace=False, _collective=True):
    nc = build(collective=_collective)
    in_maps = make_in_maps(np.asarray(x), np.asarray(Wq), np.asarray(Wkv),
                           np.asarray(Wout))
    res = bass_utils.run_bass_kernel_spmd(
        nc, in_maps, core_ids=list(range(N_CORES)), trace=_trace)
    out = np.empty((NTOK, DIM), np.float32)
    for c in range(N_CORES):
        out[c * CH:(c + 1) * CH, :] = res.results[c]["outT"].T
    full = out.reshape(B, SEQ, DIM)
    if _trace:
        return full, res
    return full


# revision 13
# speedup vs baseline: 2.5659x; 2.5659x over previous
"""CosineAttention on 8 TRN2 NeuronCores.

Sharding: head-parallel attention + AllToAll shard-transpose +
token-parallel out-projection.

  core c owns head h=c for both batches:
    - stage 1 (per 512-token chunk): [q;k]T and vT via weight-stationary
      bf16 matmuls over xT; vT DMA-XBAR-transposed (ACT queue) into packed
      [j, 64] tiles -> [j, 65] vo tiles (ones col 64); k remapped to
      partitions 0-63; per-j k sum-of-squares via ones-matmul into PSUM;
      per-token q sum-of-squares via GpSimd partition-reduce.
    - per-batch norm: rq = 1/(sqrt(mean q^2)+eps) batched over 2048
      tokens; rk = 1/(sqrt(sum k^2)+8eps) per j (folds SCALE=1/8);
      qn = q * partition_broadcast(rq).
    - phase 2 (per 512-token i-chunk): simT[j,i] = kraw^T qn; exp on ACT
      with per-partition scale AP rk[j]; attn@[v|1] accumulates so row 64
      is the softmax denominator Z; oc = av[0:64] * partition_broadcast
      (1/Z) in bf16.  Stage-1 chunks of batch 1 interleave with batch-0
      attention to keep all engines busy.
    - ONE AllToAll [512, 512] bf16: shard g = [64, 512] feature tile for
      token block g -> core receives all 512 features for its own 512
      tokens.
    - token-parallel out-proj with the full Wout (16 matmuls) -> outT
      [512 features, 512 tokens] f32; host concatenates token blocks.
"""

import numpy as np
import ml_dtypes

import concourse.bass as bass
import concourse.tile as tile
from concourse import bacc
import concourse.mybir as mybir
from concourse import bass_utils

f32 = mybir.dt.float32
f32r = mybir.dt.float32r
bf16 = mybir.dt.bfloat16
AF = mybir.ActivationFunctionType
ALU = mybir.AluOpType

N_CORES = 8
HEADS = 8
D = 64            # head dim
B = 2             # batch
SEQ = 2048        # tokens per batch
DIM = 512         # model dim
NTOK = B * SEQ    # 4096
EPS = 1e-4
SCALE = D ** -0.5  # 0.125

CH = 512          # token chunk = A2A shard = out-proj block
NCH = NTOK // CH  # 8
JPC = CH // 128   # 4 j-tiles per chunk
JPB = SEQ // 128  # 16 j-tiles per batch

_BUILD_CACHE = {}


def build(num_devices=N_CORES, collective=True):
    key = (num_devices, collective)
    if key in _BUILD_CACHE:
        return _BUILD_CACHE[key]
    nc = bacc.Bacc("TRN2", target_bir_lowering=False, debug=False,
                   num_devices=num_devices)
    xT = nc.dram_tensor("xT", [DIM, NTOK], bf16, kind="ExternalInput").ap()
    wqk = nc.dram_tensor("wqk", [DIM, 128], bf16, kind="ExternalInput").ap()
    wv = nc.dram_tensor("wv", [DIM, D], bf16, kind="ExternalInput").ap()
    w2 = nc.dram_tensor("w2", [DIM, DIM], bf16, kind="ExternalInput").ap()
    o64 = nc.dram_tensor("o64", [D, 1], bf16, kind="ExternalInput").ap()
    outT = nc.dram_tensor("outT", [DIM, CH], f32, kind="ExternalOutput").ap()

    xTr = xT.rearrange("(t p) n -> p t n", p=128)
    w2r = w2.rearrange("(t p) n -> p t n", p=128)
    wqkr = wqk.rearrange("(t p) m -> p t m", p=128)
    wvr = wv.rearrange("(t p) m -> p t m", p=128)
    outTr = outT.rearrange("(mt p) n -> p mt n", p=128)

    with tile.TileContext(nc) as tc:
        with (
            tc.tile_pool(name="persist", bufs=1) as pp,
            tc.tile_pool(name="sb", bufs=2) as sb,
            tc.tile_pool(name="ps", bufs=1, space="PSUM") as ps,
            tc.tile_pool(name="dram", bufs=1, space="DRAM") as dram,
            nc.allow_low_precision(reason="bf16 matmul path"),
        ):
            # ---- persistent weights / constants ----
            wqk_sb = pp.tile([128, 4, 128], bf16)
            wv_sb = pp.tile([128, 4, D], bf16)
            w2_sb = pp.tile([128, 4, DIM], bf16)
            nc.sync.dma_start(wqk_sb[:], wqkr[:])
            nc.sync.dma_start(wv_sb[:], wvr[:])
            nc.sync.dma_start(w2_sb[:], w2r[:])
            o64_sb = pp.tile([D, 1], bf16)
            nc.sync.dma_start(o64_sb[:], o64[:])

            # ---- persistent activations ----
            qk_all = pp.tile([128, NTOK], bf16)   # [qT; kT] raw
            qn_sb = pp.tile([D, NTOK], bf16)      # normalized qT
            kraw_sb = pp.tile([D, NTOK], bf16)    # raw kT at partitions 0-63
            vo_sb = pp.tile([128, NCH * JPC, D + 1], bf16)  # [v | ones]
            rtq_sb = pp.tile([1, NTOK], f32)      # sqrt(mean q^2) per token
            rks_sb = pp.tile([128, NCH * JPC], f32)  # 1/(|k|+8eps) per j
            nc.gpsimd.memset(vo_sb[:, :, D:D + 1], 1.0)

            cc_in = dram.tile([DIM, CH], bf16, name="cc_in")
            cc_out = dram.tile([DIM, CH], bf16, name="cc_out")

            stk_ps = ps.tile([128, NCH * JPC], f32, tag="stk", bufs=1)

            def stage1_chunk(ci):
                cols = slice(ci * CH, (ci + 1) * CH)
                xt = sb.tile([128, 4, CH], bf16, tag="xt")
                nc.sync.dma_start(xt[:], xTr[:, :, cols])
                qk_ps = ps.tile([128, CH], f32, tag="a", bufs=3)
                for t in range(4):
                    nc.tensor.matmul(qk_ps[:], wqk_sb[:, t, :], xt[:, t, :],
                                     start=(t == 0), stop=(t == 3))
                vt_ps = ps.tile([D, CH], f32, tag="b", bufs=2)
                for t in range(4):
                    nc.tensor.matmul(vt_ps[:], wv_sb[:, t, :], xt[:, t, :],
                                     start=(t == 0), stop=(t == 3))
                nc.vector.tensor_copy(qk_all[:, cols], qk_ps[:])
                # raw kT at partitions 0-63 (DMA partition remap, ACT queue)
                nc.scalar.dma_start(kraw_sb[:, cols], qk_all[64:128, cols])
                # vT -> packed [j, d] tiles via DMA XBAR transpose (ACT queue)
                vt_sb = sb.tile([D, CH], bf16, tag="vtsb")
                nc.vector.tensor_copy(vt_sb[:], vt_ps[:])
                for jj in range(JPC):
                    jt = ci * JPC + jj
                    js = slice(jj * 128, (jj + 1) * 128)
                    vtr = sb.tile([128, D], bf16, tag="vtr", bufs=4)
                    nc.scalar.dma_start_transpose(vtr[:], vt_sb[:, js])
                    nc.gpsimd.tensor_copy(vo_sb[:, jt, 0:D], vtr[:])
                # k sum of squares per j (ones-matmul on remapped kraw)
                ksq = sb.tile([D, CH], bf16, tag="ksq")
                nc.vector.tensor_mul(ksq[:], kraw_sb[:, cols], kraw_sb[:, cols])
                for jj in range(JPC):
                    jt = ci * JPC + jj
                    js = slice(jj * 128, (jj + 1) * 128)
                    nc.tensor.matmul(stk_ps[:, jt:jt + 1], ksq[:, js],
                                     o64_sb[:], start=True, stop=True)
                # q sum of squares per token (GpSimd partition reduce)
                sq_q = sb.tile([D, CH], bf16, tag="sqq")
                nc.vector.tensor_mul(sq_q[:], qk_all[0:D, cols],
                                     qk_all[0:D, cols])
                stq_ps = ps.tile([1, CH], f32, tag="s", bufs=2)
                nc.tensor.matmul(stq_ps[:], o64_sb[:], sq_q[:],
                                 start=True, stop=True)
                nc.scalar.activation(rtq_sb[:, cols], stq_ps[:], AF.Sqrt,
                                     scale=1.0 / D)

            def norm_half(h):
                # k reciprocal for batch h
                hs = slice(h * JPB, (h + 1) * JPB)
                skh = sb.tile([128, JPB], f32, tag="skh")
                nc.scalar.activation(skh[:], stk_ps[:, hs], AF.Sqrt)
                seh = sb.tile([128, JPB], f32, tag="seh")
                nc.vector.tensor_scalar_add(seh[:], skh[:], 8.0 * EPS)
                nc.vector.reciprocal(rks_sb[:, hs], seh[:])
                # q normalization for batch h, batched over 2048 tokens
                tok = slice(h * SEQ, (h + 1) * SEQ)
                req = sb.tile([1, SEQ], f32, tag="req")
                nc.vector.tensor_scalar_add(req[:], rtq_sb[:, tok], EPS)
                rcq = sb.tile([1, SEQ], f32, tag="rcq")
                nc.vector.reciprocal(rcq[:], req[:])
                rcq_b = sb.tile([1, SEQ], bf16, tag="rcqb")
                nc.vector.tensor_copy(rcq_b[:], rcq[:])
                for cj in range(NCH // B):
                    cols = slice(h * SEQ + cj * CH, h * SEQ + (cj + 1) * CH)
                    rb_sb = sb.tile([D, CH], bf16, tag="rbsb")
                    nc.gpsimd.partition_broadcast(
                        rb_sb[:], rcq_b[:, cj * CH:(cj + 1) * CH])
                    nc.vector.tensor_mul(qn_sb[:, cols], qk_all[0:D, cols],
                                         rb_sb[:])

            def phase2_chunk(b, ch):
                g = b * (NCH // B) + ch
                i0 = g * CH
                expT = sb.tile([128, JPB, CH], bf16, tag="exp")
                for jt in range(JPB):
                    j0 = b * SEQ + jt * 128
                    sim_ps = ps.tile([128, CH], f32, tag="a", bufs=3)
                    nc.tensor.matmul(sim_ps[:], kraw_sb[:, j0:j0 + 128],
                                     qn_sb[:, i0:i0 + CH],
                                     start=True, stop=True)
                    gj = b * JPB + jt
                    nc.scalar.activation(expT[:, jt, :], sim_ps[:], AF.Exp,
                                         scale=rks_sb[:, gj:gj + 1])
                av_ps = ps.tile([D + 1, CH], f32, tag="b", bufs=2)
                for jt in range(JPB):
                    nc.tensor.matmul(av_ps[:], vo_sb[:, b * JPB + jt, :],
                                     expT[:, jt, :],
                                     start=(jt == 0), stop=(jt == JPB - 1))
                rse = sb.tile([1, CH], f32, tag="rse")
                nc.vector.reciprocal(rse[:], av_ps[D:D + 1, :])
                r2_sb = sb.tile([D, CH], f32, tag="r2sb")
                nc.gpsimd.partition_broadcast(r2_sb[:], rse[:])
                oc = sb.tile([D, CH], bf16, tag="oc")
                nc.vector.tensor_mul(oc[:], av_ps[0:D, :], r2_sb[:])
                nc.sync.dma_start(cc_in[g * D:(g + 1) * D, :], oc[:])

            # ---- schedule ----
            for ci in range(4):
                stage1_chunk(ci)
            norm_half(0)
            for ch in range(4):
                stage1_chunk(4 + ch)
                phase2_chunk(0, ch)
            norm_half(1)
            for ch in range(4):
                phase2_chunk(1, ch)

            # ---- shard transpose: one AllToAll ----
            if collective:
                nc.gpsimd.collective_compute(
                    "AllToAll", ALU.bypass,
                    replica_groups=[list(range(num_devices))],
                    ins=[cc_in[:]], outs=[cc_out[:]])
            else:
                # timing-only stand-in (numerically wrong off-diagonal)
                nc.sync.dma_start(cc_out[:], cc_in[:])

            # ---- token-parallel out-projection ----
            ag = sb.tile([128, 4, CH], bf16, tag="ag")
            nc.sync.dma_start(ag[:], cc_out.rearrange("(t p) n -> p t n",
                                                      p=128))
            fo = sb.tile([128, 4, CH], f32, tag="fo")
            for mt in range(4):
                fp_ps = ps.tile([128, CH], f32, tag="a", bufs=3)
                for t in range(4):
                    nc.tensor.matmul(fp_ps[:],
                                     w2_sb[:, t, mt * 128:(mt + 1) * 128],
                                     ag[:, t, :], start=(t == 0), stop=(t == 3))
                nc.vector.tensor_copy(fo[:, mt, :], fp_ps[:])
            nc.sync.dma_start(outTr[:], fo[:])
    nc.compile()
    _BUILD_CACHE[key] = nc
    return nc


def make_in_maps(x, Wq, Wkv, Wout):
    xT = np.ascontiguousarray(
        x.reshape(NTOK, DIM).T).astype(ml_dtypes.bfloat16)
    w2 = np.ascontiguousarray(Wout.T).astype(ml_dtypes.bfloat16)
    o64 = np.ones((D, 1), ml_dtypes.bfloat16)
    in_maps = []
    for c in range(N_CORES):
        rows = slice(c * D, (c + 1) * D)
        wqk = np.ascontiguousarray(
            np.concatenate([Wq[rows, :].T, Wkv[rows, :].T],
                           axis=1)).astype(ml_dtypes.bfloat16)
        wv = np.ascontiguousarray(
            Wkv[DIM + c * D:DIM + (c + 1) * D, :].T).astype(ml_dtypes.bfloat16)
        in_maps.append({
            "xT": xT, "wqk": wqk, "wv": wv, "w2": w2, "o64": o64,
        })
    return in_maps


def kernel(x, Wq, Wkv, Wout, _trace=False, _collective=True):
    nc = build(collective=_collective)
    in_maps = make_in_maps(np.asarray(x), np.asarray(Wq), np.asarray(Wkv),
                           np.asarray(Wout))
    res = bass_utils.run_bass_kernel_spmd(
        nc, in_maps, core_ids=list(range(N_CORES)), trace=_trace)
    out = np.empty((NTOK, DIM), np.float32)
    for c in range(N_CORES):
        out[c * CH:(c + 1) * CH, :] = res.results[c]["outT"].T
    full = out.reshape(B, SEQ, DIM)
    if _trace:
        return full, res
    return full


# revision 17
# speedup vs baseline: 2.7177x; 1.0591x over previous
"""CosineAttention on 8 TRN2 NeuronCores.

Sharding: head-parallel attention + AllToAll shard-transpose +
token-parallel out-projection.

  core c owns head h=c for both batches:
    - stage 1 (per 512-token chunk): [q;k]T and vT via weight-stationary
      bf16 matmuls over xT; vT DMA-XBAR-transposed (ACT queue) into packed
      [j, 64] tiles -> [j, 65] vo tiles (ones col 64); k remapped to
      partitions 0-63; per-j k sum-of-squares via ones-matmul into PSUM;
      per-token q sum-of-squares via GpSimd partition-reduce.
    - per-batch norm: rq = 1/(sqrt(mean q^2)+eps) batched over 2048
      tokens; rk = 1/(sqrt(sum k^2)+8eps) per j (folds SCALE=1/8);
      qn = q * partition_broadcast(rq).
    - phase 2 (per 512-token i-chunk): simT[j,i] = kraw^T qn; exp on ACT
      with per-partition scale AP rk[j]; attn@[v|1] accumulates so row 64
      is the softmax denominator Z; oc = av[0:64] * partition_broadcast
      (1/Z) in bf16.  Stage-1 chunks of batch 1 interleave with batch-0
      attention to keep all engines busy.
    - ONE AllToAll [512, 512] bf16: shard g = [64, 512] feature tile for
      token block g -> core receives all 512 features for its own 512
      tokens.
    - token-parallel out-proj with the full Wout (16 matmuls) -> outT
      [512 features, 512 tokens] f32; host concatenates token blocks.
"""

import numpy as np
import ml_dtypes

import concourse.bass as bass
import concourse.tile as tile
from concourse import bacc
import concourse.mybir as mybir
from concourse import bass_utils

f32 = mybir.dt.float32
f32r = mybir.dt.float32r
bf16 = mybir.dt.bfloat16
AF = mybir.ActivationFunctionType
ALU = mybir.AluOpType

N_CORES = 8
HEADS = 8
D = 64            # head dim
B = 2             # batch
SEQ = 2048        # tokens per batch
DIM = 512         # model dim
NTOK = B * SEQ    # 4096
EPS = 1e-4
SCALE = D ** -0.5  # 0.125

CH = 512          # token chunk = A2A shard = out-proj block
NCH = NTOK // CH  # 8
JPC = CH // 128   # 4 j-tiles per chunk
JPB = SEQ // 128  # 16 j-tiles per batch

_BUILD_CACHE = {}


def build(num_devices=N_CORES, collective=True):
    key = (num_devices, collective)
    if key in _BUILD_CACHE:
        return _BUILD_CACHE[key]
    nc = bacc.Bacc("TRN2", target_bir_lowering=False, debug=False,
                   num_devices=num_devices)
    xT = nc.dram_tensor("xT", [DIM, NTOK], bf16, kind="ExternalInput").ap()
    wqk = nc.dram_tensor("wqk", [DIM, 128], bf16, kind="ExternalInput").ap()
    wv = nc.dram_tensor("wv", [DIM, D], bf16, kind="ExternalInput").ap()
    w2 = nc.dram_tensor("w2", [DIM, DIM], bf16, kind="ExternalInput").ap()
    o64 = nc.dram_tensor("o64", [D, 1], bf16, kind="ExternalInput").ap()
    outT = nc.dram_tensor("outT", [DIM, CH], f32, kind="ExternalOutput").ap()

    xTr = xT.rearrange("(t p) n -> p t n", p=128)
    w2r = w2.rearrange("(t p) n -> p t n", p=128)
    wqkr = wqk.rearrange("(t p) m -> p t m", p=128)
    wvr = wv.rearrange("(t p) m -> p t m", p=128)
    outTr = outT.rearrange("(mt p) n -> p mt n", p=128)

    with tile.TileContext(nc) as tc:
        with (
            tc.tile_pool(name="persist", bufs=1) as pp,
            tc.tile_pool(name="sb", bufs=2) as sb,
            tc.tile_pool(name="ps", bufs=1, space="PSUM") as ps,
            tc.tile_pool(name="dram", bufs=1, space="DRAM") as dram,
            nc.allow_low_precision(reason="bf16 matmul path"),
        ):
            # ---- persistent weights / constants ----
            wqk_sb = pp.tile([128, 4, 128], bf16)
            wv_sb = pp.tile([128, 4, D], bf16)
            w2_sb = pp.tile([128, 4, DIM], bf16)
            nc.sync.dma_start(wqk_sb[:], wqkr[:])
            nc.sync.dma_start(wv_sb[:], wvr[:])
            nc.sync.dma_start(w2_sb[:], w2r[:])
            o64_sb = pp.tile([D, 1], bf16)
            nc.sync.dma_start(o64_sb[:], o64[:])

            # ---- persistent activations ----
            qk_all = pp.tile([128, NTOK], bf16)   # [qT; kT] raw
            qn_sb = pp.tile([D, NTOK], bf16)      # normalized qT
            kraw_sb = pp.tile([D, NTOK], bf16)    # raw kT at partitions 0-63
            vo_sb = pp.tile([128, NCH * JPC, D + 1], bf16)  # [v | ones]
            rtq_sb = pp.tile([1, NTOK], f32)      # sqrt(mean q^2) per token
            rks_sb = pp.tile([128, NCH * JPC], f32)  # 1/(|k|+8eps) per j
            nc.gpsimd.memset(vo_sb[:, :, D:D + 1], 1.0)

            cc_in = dram.tile([DIM, CH], bf16, name="cc_in")
            cc_out = dram.tile([DIM, CH], bf16, name="cc_out")

            sks_sb = pp.tile([128, NCH * JPC], f32)  # sqrt(sum k^2) per j

            def stage1_chunk(ci):
                cols = slice(ci * CH, (ci + 1) * CH)
                xt = sb.tile([128, 4, CH], bf16, tag="xt")
                nc.sync.dma_start(xt[:], xTr[:, :, cols])
                qk_ps = ps.tile([128, CH], f32, tag="a", bufs=3)
                for t in range(4):
                    nc.tensor.matmul(qk_ps[:], wqk_sb[:, t, :], xt[:, t, :],
                                     start=(t == 0), stop=(t == 3))
                vt_ps = ps.tile([D, CH], f32, tag="b", bufs=2)
                for t in range(4):
                    nc.tensor.matmul(vt_ps[:], wv_sb[:, t, :], xt[:, t, :],
                                     start=(t == 0), stop=(t == 3))
                nc.vector.tensor_copy(qk_all[:, cols], qk_ps[:])
                # raw kT at partitions 0-63 (DMA partition remap, ACT queue)
                nc.scalar.dma_start(kraw_sb[:, cols], qk_all[64:128, cols])
                # vT -> packed [j, d] tiles via DMA XBAR transpose (ACT queue)
                vt_sb = sb.tile([D, CH], bf16, tag="vtsb")
                nc.vector.tensor_copy(vt_sb[:], vt_ps[:])
                for jj in range(JPC):
                    jt = ci * JPC + jj
                    js = slice(jj * 128, (jj + 1) * 128)
                    vtr = sb.tile([128, D], bf16, tag="vtr", bufs=4)
                    nc.scalar.dma_start_transpose(vtr[:], vt_sb[:, js])
                    nc.gpsimd.tensor_copy(vo_sb[:, jt, 0:D], vtr[:])
                # k sum of squares per j (ones-matmul on remapped kraw)
                ksq = sb.tile([D, CH], bf16, tag="ksq")
                nc.vector.tensor_mul(ksq[:], kraw_sb[:, cols], kraw_sb[:, cols])
                stk_ps = ps.tile([128, JPC], f32, tag="k", bufs=1)
                for jj in range(JPC):
                    js = slice(jj * 128, (jj + 1) * 128)
                    nc.tensor.matmul(stk_ps[:, jj:jj + 1], ksq[:, js],
                                     o64_sb[:], start=True, stop=True)
                nc.scalar.activation(sks_sb[:, ci * JPC:(ci + 1) * JPC],
                                     stk_ps[:], AF.Sqrt)
                # q sum of squares per token (GpSimd partition reduce)
                sq_q = sb.tile([D, CH], bf16, tag="sqq")
                nc.vector.tensor_mul(sq_q[:], qk_all[0:D, cols],
                                     qk_all[0:D, cols])
                stq_ps = ps.tile([1, CH], f32, tag="s", bufs=1)
                nc.tensor.matmul(stq_ps[:], o64_sb[:], sq_q[:],
                                 start=True, stop=True)
                nc.scalar.activation(rtq_sb[:, cols], stq_ps[:], AF.Sqrt,
                                     scale=1.0 / D)

            def norm_half(h):
                # k reciprocal for batch h
                hs = slice(h * JPB, (h + 1) * JPB)
                seh = sb.tile([128, JPB], f32, tag="seh")
                nc.vector.tensor_scalar_add(seh[:], sks_sb[:, hs], 8.0 * EPS)
                nc.vector.reciprocal(rks_sb[:, hs], seh[:])
                # q normalization for batch h, batched over 2048 tokens
                tok = slice(h * SEQ, (h + 1) * SEQ)
                req = sb.tile([1, SEQ], f32, tag="req")
                nc.vector.tensor_scalar_add(req[:], rtq_sb[:, tok], EPS)
                rcq = sb.tile([1, SEQ], f32, tag="rcq")
                nc.vector.reciprocal(rcq[:], req[:])
                rcq_b = sb.tile([1, SEQ], bf16, tag="rcqb")
                nc.vector.tensor_copy(rcq_b[:], rcq[:])
                for cj in range(NCH // B):
                    cols = slice(h * SEQ + cj * CH, h * SEQ + (cj + 1) * CH)
                    rb_sb = sb.tile([D, CH], bf16, tag="rbsb")
                    nc.gpsimd.partition_broadcast(
                        rb_sb[:], rcq_b[:, cj * CH:(cj + 1) * CH])
                    nc.vector.tensor_mul(qn_sb[:, cols], qk_all[0:D, cols],
                                         rb_sb[:])

            def phase2_chunk(b, ch):
                g = b * (NCH // B) + ch
                i0 = g * CH
                expT = sb.tile([128, JPB, CH], bf16, tag="exp")
                for jt in range(JPB):
                    j0 = b * SEQ + jt * 128
                    sim_ps = ps.tile([128, CH], f32, tag="a", bufs=3)
                    nc.tensor.matmul(sim_ps[:], kraw_sb[:, j0:j0 + 128],
                                     qn_sb[:, i0:i0 + CH],
                                     start=True, stop=True)
                    gj = b * JPB + jt
                    nc.scalar.activation(expT[:, jt, :], sim_ps[:], AF.Exp,
                                         scale=rks_sb[:, gj:gj + 1])
                av_ps = ps.tile([D + 1, CH], f32, tag="b", bufs=2)
                for jt in range(JPB):
                    nc.tensor.matmul(av_ps[:], vo_sb[:, b * JPB + jt, :],
                                     expT[:, jt, :],
                                     start=(jt == 0), stop=(jt == JPB - 1))
                rse = sb.tile([1, CH], f32, tag="rse")
                nc.vector.reciprocal(rse[:], av_ps[D:D + 1, :])
                r2_sb = sb.tile([D, CH], f32, tag="r2sb")
                nc.gpsimd.partition_broadcast(r2_sb[:], rse[:])
                oc = sb.tile([D, CH], bf16, tag="oc")
                nc.vector.tensor_mul(oc[:], av_ps[0:D, :], r2_sb[:])
                nc.sync.dma_start(cc_in[g * D:(g + 1) * D, :], oc[:])

            # ---- schedule ----
            for ci in range(4):
                stage1_chunk(ci)
            norm_half(0)
            for ci in range(4, 8):
                stage1_chunk(ci)
            norm_half(1)
            for ch in range(4):
                phase2_chunk(0, ch)
            for ch in range(4):
                phase2_chunk(1, ch)

            # ---- shard transpose: one AllToAll ----
            if collective:
                nc.gpsimd.collective_compute(
                    "AllToAll", ALU.bypass,
                    replica_groups=[list(range(num_devices))],
                    ins=[cc_in[:]], outs=[cc_out[:]])
            else:
                # timing-only stand-in (numerically wrong off-diagonal)
                nc.sync.dma_start(cc_out[:], cc_in[:])

            # ---- token-parallel out-projection ----
            ag = sb.tile([128, 4, CH], bf16, tag="ag")
            nc.sync.dma_start(ag[:], cc_out.rearrange("(t p) n -> p t n",
                                                      p=128))
            fo = sb.tile([128, 4, CH], f32, tag="fo")
            for mt in range(4):
                fp_ps = ps.tile([128, CH], f32, tag="a", bufs=3)
                for t in range(4):
                    nc.tensor.matmul(fp_ps[:],
                                     w2_sb[:, t, mt * 128:(mt + 1) * 128],
                                     ag[:, t, :], start=(t == 0), stop=(t == 3))
                nc.vector.tensor_copy(fo[:, mt, :], fp_ps[:])
            nc.sync.dma_start(outTr[:], fo[:])
    nc.compile()
    _BUILD_CACHE[key] = nc
    return nc


def make_in_maps(x, Wq, Wkv, Wout):
    xT = np.ascontiguousarray(
        x.reshape(NTOK, DIM).T).astype(ml_dtypes.bfloat16)
    w2 = np.ascontiguousarray(Wout.T).astype(ml_dtypes.bfloat16)
    o64 = np.ones((D, 1), ml_dtypes.bfloat16)
    in_maps = []
    for c in range(N_CORES):
        rows = slice(c * D, (c + 1) * D)
        wqk = np.ascontiguousarray(
            np.concatenate([Wq[rows, :].T, Wkv[rows, :].T],
                           axis=1)).astype(ml_dtypes.bfloat16)
        wv = np.ascontiguousarray(
            Wkv[DIM + c * D:DIM + (c + 1) * D, :].T).astype(ml_dtypes.bfloat16)
        in_maps.append({
            "xT": xT, "wqk": wqk, "wv": wv, "w2": w2, "o64": o64,
        })
    return in_maps


def kernel(x, Wq, Wkv, Wout, _trace=False, _collective=True):
    nc = build(collective=_collective)
    in_maps = make_in_maps(np.asarray(x), np.asarray(Wq), np.asarray(Wkv),
                           np.asarray(Wout))
    res = bass_utils.run_bass_kernel_spmd(
        nc, in_maps, core_ids=list(range(N_CORES)), trace=_trace)
    out = np.empty((NTOK, DIM), np.float32)
    for c in range(N_CORES):
        out[c * CH:(c + 1) * CH, :] = res.results[c]["outT"].T
    full = out.reshape(B, SEQ, DIM)
    if _trace:
        return full, res
    return full


# revision 19
# speedup vs baseline: 2.7473x; 1.0109x over previous
"""CosineAttention on 8 TRN2 NeuronCores.

Sharding: head-parallel attention + AllToAll shard-transpose +
token-parallel out-projection.

  core c owns head h=c for both batches:
    - stage 1 (per 512-token chunk): [q;k]T and vT via weight-stationary
      bf16 matmuls over xT; vT DMA-XBAR-transposed (ACT queue) into packed
      [j, 64] tiles -> [j, 65] vo tiles (ones col 64); k remapped to
      partitions 0-63; per-j k sum-of-squares via ones-matmul into PSUM;
      per-token q sum-of-squares via GpSimd partition-reduce.
    - per-batch norm: rq = 1/(sqrt(mean q^2)+eps) batched over 2048
      tokens; rk = 1/(sqrt(sum k^2)+8eps) per j (folds SCALE=1/8);
      qn = q * partition_broadcast(rq).
    - phase 2 (per 512-token i-chunk): simT[j,i] = kraw^T qn; exp on ACT
      with per-partition scale AP rk[j]; attn@[v|1] accumulates so row 64
      is the softmax denominator Z; oc = av[0:64] * partition_broadcast
      (1/Z) in bf16.  Stage-1 chunks of batch 1 interleave with batch-0
      attention to keep all engines busy.
    - ONE AllToAll [512, 512] bf16: shard g = [64, 512] feature tile for
      token block g -> core receives all 512 features for its own 512
      tokens.
    - token-parallel out-proj with the full Wout (16 matmuls) -> outT
      [512 features, 512 tokens] f32; host concatenates token blocks.
"""

import numpy as np
import ml_dtypes

import concourse.bass as bass
import concourse.tile as tile
from concourse import bacc
import concourse.mybir as mybir
from concourse import bass_utils

f32 = mybir.dt.float32
f32r = mybir.dt.float32r
bf16 = mybir.dt.bfloat16
AF = mybir.ActivationFunctionType
ALU = mybir.AluOpType

N_CORES = 8
HEADS = 8
D = 64            # head dim
B = 2             # batch
SEQ = 2048        # tokens per batch
DIM = 512         # model dim
NTOK = B * SEQ    # 4096
EPS = 1e-4
SCALE = D ** -0.5  # 0.125

CH = 512          # token chunk = A2A shard = out-proj block
NCH = NTOK // CH  # 8
JPC = CH // 128   # 4 j-tiles per chunk
JPB = SEQ // 128  # 16 j-tiles per batch

_BUILD_CACHE = {}


def build(num_devices=N_CORES, collective=True):
    key = (num_devices, collective)
    if key in _BUILD_CACHE:
        return _BUILD_CACHE[key]
    nc = bacc.Bacc("TRN2", target_bir_lowering=False, debug=False,
                   num_devices=num_devices)
    xT = nc.dram_tensor("xT", [DIM, NTOK], bf16, kind="ExternalInput").ap()
    wqk = nc.dram_tensor("wqk", [DIM, 128], bf16, kind="ExternalInput").ap()
    wv = nc.dram_tensor("wv", [DIM, D], bf16, kind="ExternalInput").ap()
    w2 = nc.dram_tensor("w2", [DIM, DIM], bf16, kind="ExternalInput").ap()
    o64 = nc.dram_tensor("o64", [D, 1], bf16, kind="ExternalInput").ap()
    outT = nc.dram_tensor("outT", [DIM, CH], f32, kind="ExternalOutput").ap()

    xTr = xT.rearrange("(t p) n -> p t n", p=128)
    w2r = w2.rearrange("(t p) n -> p t n", p=128)
    wqkr = wqk.rearrange("(t p) m -> p t m", p=128)
    wvr = wv.rearrange("(t p) m -> p t m", p=128)
    outTr = outT.rearrange("(mt p) n -> p mt n", p=128)

    with tile.TileContext(nc) as tc:
        with (
            tc.tile_pool(name="persist", bufs=1) as pp,
            tc.tile_pool(name="sb", bufs=2) as sb,
            tc.tile_pool(name="ps", bufs=1, space="PSUM") as ps,
            tc.tile_pool(name="dram", bufs=1, space="DRAM") as dram,
            nc.allow_low_precision(reason="bf16 matmul path"),
        ):
            # ---- persistent weights / constants ----
            wqk_sb = pp.tile([128, 4, 128], bf16)
            wv_sb = pp.tile([128, 4, D], bf16)
            w2_sb = pp.tile([128, 4, DIM], bf16)
            nc.sync.dma_start(wqk_sb[:], wqkr[:])
            nc.sync.dma_start(wv_sb[:], wvr[:])
            nc.sync.dma_start(w2_sb[:], w2r[:])
            o64_sb = pp.tile([D, 1], bf16)
            nc.sync.dma_start(o64_sb[:], o64[:])

            # ---- persistent activations ----
            qk_all = pp.tile([128, NTOK], bf16)   # [qT; kT] raw
            qn_sb = pp.tile([D, NTOK], bf16)      # normalized qT
            kraw_sb = pp.tile([D, NTOK], bf16)    # raw kT at partitions 0-63
            vo_sb = pp.tile([128, NCH * JPC, D + 1], bf16)  # [v | ones]
            rtq_sb = pp.tile([1, NTOK], f32)      # sqrt(mean q^2) per token
            rks_sb = pp.tile([128, NCH * JPC], f32)  # 1/(|k|+8eps) per j
            nc.gpsimd.memset(vo_sb[:, :, D:D + 1], 1.0)

            cc_in = dram.tile([DIM, CH], bf16, name="cc_in")
            cc_out = dram.tile([DIM, CH], bf16, name="cc_out")

            sks_sb = pp.tile([128, NCH * JPC], f32)  # sqrt(sum k^2) per j

            def stage1_chunk(ci):
                cols = slice(ci * CH, (ci + 1) * CH)
                xt = sb.tile([128, 4, CH], bf16, tag="xt")
                nc.sync.dma_start(xt[:], xTr[:, :, cols])
                qk_ps = ps.tile([128, CH], f32, tag="a", bufs=3)
                for t in range(4):
                    nc.tensor.matmul(qk_ps[:], wqk_sb[:, t, :], xt[:, t, :],
                                     start=(t == 0), stop=(t == 3))
                vt_ps = ps.tile([D, CH], f32, tag="b", bufs=2)
                for t in range(4):
                    nc.tensor.matmul(vt_ps[:], wv_sb[:, t, :], xt[:, t, :],
                                     start=(t == 0), stop=(t == 3))
                nc.vector.tensor_copy(qk_all[:, cols], qk_ps[:])
                # raw kT at partitions 0-63 (DMA partition remap, ACT queue)
                nc.scalar.dma_start(kraw_sb[:, cols], qk_all[64:128, cols])
                # vT -> packed [j, d] tiles via DMA XBAR transpose (ACT queue)
                vt_sb = sb.tile([D, CH], bf16, tag="vtsb")
                nc.vector.tensor_copy(vt_sb[:], vt_ps[:])
                for jj in range(JPC):
                    jt = ci * JPC + jj
                    js = slice(jj * 128, (jj + 1) * 128)
                    vtr = sb.tile([128, D], bf16, tag="vtr", bufs=4)
                    nc.scalar.dma_start_transpose(vtr[:], vt_sb[:, js])
                    nc.gpsimd.tensor_copy(vo_sb[:, jt, 0:D], vtr[:])
                # k sum of squares per j (ones-matmul on remapped kraw)
                ksq = sb.tile([D, CH], bf16, tag="ksq")
                nc.vector.tensor_mul(ksq[:], kraw_sb[:, cols], kraw_sb[:, cols])
                stk_ps = ps.tile([128, JPC], f32, tag="k", bufs=1)
                for jj in range(JPC):
                    js = slice(jj * 128, (jj + 1) * 128)
                    nc.tensor.matmul(stk_ps[:, jj:jj + 1], ksq[:, js],
                                     o64_sb[:], start=True, stop=True)
                nc.scalar.activation(sks_sb[:, ci * JPC:(ci + 1) * JPC],
                                     stk_ps[:], AF.Sqrt)
                # q sum of squares per token (GpSimd partition reduce)
                sq_q = sb.tile([D, CH], bf16, tag="sqq")
                nc.vector.tensor_mul(sq_q[:], qk_all[0:D, cols],
                                     qk_all[0:D, cols])
                stq_ps = ps.tile([1, CH], f32, tag="s", bufs=1)
                nc.tensor.matmul(stq_ps[:], o64_sb[:], sq_q[:],
                                 start=True, stop=True)
                nc.scalar.activation(rtq_sb[:, cols], stq_ps[:], AF.Sqrt,
                                     scale=1.0 / D)

            def norm_half(h):
                # k reciprocal for batch h
                hs = slice(h * JPB, (h + 1) * JPB)
                seh = sb.tile([128, JPB], f32, tag="seh")
                nc.vector.tensor_scalar_add(seh[:], sks_sb[:, hs], 8.0 * EPS)
                nc.vector.reciprocal(rks_sb[:, hs], seh[:])
                # q normalization for batch h, batched over 2048 tokens
                tok = slice(h * SEQ, (h + 1) * SEQ)
                req = sb.tile([1, SEQ], f32, tag="req")
                nc.vector.tensor_scalar_add(req[:], rtq_sb[:, tok], EPS)
                rcq = sb.tile([1, SEQ], f32, tag="rcq")
                nc.vector.reciprocal(rcq[:], req[:])
                rcq_b = sb.tile([1, SEQ], bf16, tag="rcqb")
                nc.vector.tensor_copy(rcq_b[:], rcq[:])
                for cj in range(NCH // B):
                    cols = slice(h * SEQ + cj * CH, h * SEQ + (cj + 1) * CH)
                    rb_sb = sb.tile([D, CH], bf16, tag="rbsb")
                    nc.gpsimd.partition_broadcast(
                        rb_sb[:], rcq_b[:, cj * CH:(cj + 1) * CH])
                    nc.vector.tensor_mul(qn_sb[:, cols], qk_all[0:D, cols],
                                         rb_sb[:])

            def phase2_chunk(b, ch):
                g = b * (NCH // B) + ch
                i0 = g * CH
                expT = sb.tile([128, JPB, CH], bf16, tag="exp")
                for jt in range(JPB):
                    j0 = b * SEQ + jt * 128
                    sim_ps = ps.tile([128, CH], f32, tag="a", bufs=3)
                    nc.tensor.matmul(sim_ps[:], kraw_sb[:, j0:j0 + 128],
                                     qn_sb[:, i0:i0 + CH],
                                     start=True, stop=True)
                    gj = b * JPB + jt
                    nc.scalar.activation(expT[:, jt, :], sim_ps[:], AF.Exp,
                                         scale=rks_sb[:, gj:gj + 1])
                av_ps = ps.tile([D + 1, CH], f32, tag="b", bufs=2)
                for jt in range(JPB):
                    nc.tensor.matmul(av_ps[:], vo_sb[:, b * JPB + jt, :],
                                     expT[:, jt, :],
                                     start=(jt == 0), stop=(jt == JPB - 1))
                rse = sb.tile([1, CH], f32, tag="rse")
                nc.vector.reciprocal(rse[:], av_ps[D:D + 1, :])
                r2_sb = sb.tile([D, CH], f32, tag="r2sb")
                nc.gpsimd.partition_broadcast(r2_sb[:], rse[:])
                oc = sb.tile([D, CH], bf16, tag="oc")
                nc.vector.tensor_mul(oc[:], av_ps[0:D, :], r2_sb[:])
                nc.sync.dma_start(cc_in[g * D:(g + 1) * D, :], oc[:])

            # ---- schedule ----
            for ci in range(4):
                stage1_chunk(ci)
            norm_half(0)
            for ch in range(4):
                stage1_chunk(4 + ch)
                phase2_chunk(0, ch)
            norm_half(1)
            for ch in range(4):
                phase2_chunk(1, ch)

            # ---- shard transpose: one AllToAll ----
            if collective:
                nc.gpsimd.collective_compute(
                    "AllToAll", ALU.bypass,
                    replica_groups=[list(range(num_devices))],
                    ins=[cc_in[:]], outs=[cc_out[:]])
            else:
                # timing-only stand-in (numerically wrong off-diagonal)
                nc.sync.dma_start(cc_out[:], cc_in[:])

            # ---- token-parallel out-projection ----
            ag = sb.tile([128, 4, CH], bf16, tag="ag")
            nc.sync.dma_start(ag[:], cc_out.rearrange("(t p) n -> p t n",
                                                      p=128))
            fo = sb.tile([128, 4, CH], f32, tag="fo")
            for mt in range(4):
                fp_ps = ps.tile([128, CH], f32, tag="a", bufs=3)
                for t in range(4):
                    nc.tensor.matmul(fp_ps[:],
                                     w2_sb[:, t, mt * 128:(mt + 1) * 128],
                                     ag[:, t, :], start=(t == 0), stop=(t == 3))
                nc.vector.tensor_copy(fo[:, mt, :], fp_ps[:])
            nc.sync.dma_start(outTr[:], fo[:])
    nc.compile()
    _BUILD_CACHE[key] = nc
    return nc


def make_in_maps(x, Wq, Wkv, Wout):
    xT = np.ascontiguousarray(
        x.reshape(NTOK, DIM).T).astype(ml_dtypes.bfloat16)
    w2 = np.ascontiguousarray(Wout.T).astype(ml_dtypes.bfloat16)
    o64 = np.ones((D, 1), ml_dtypes.bfloat16)
    in_maps = []
    for c in range(N_CORES):
        rows = slice(c * D, (c + 1) * D)
        wqk = np.ascontiguousarray(
            np.concatenate([Wq[rows, :].T, Wkv[rows, :].T],
                           axis=1)).astype(ml_dtypes.bfloat16)
        wv = np.ascontiguousarray(
            Wkv[DIM + c * D:DIM + (c + 1) * D, :].T).astype(ml_dtypes.bfloat16)
        in_maps.append({
            "xT": xT, "wqk": wqk, "wv": wv, "w2": w2, "o64": o64,
        })
    return in_maps


def kernel(x, Wq, Wkv, Wout, _trace=False, _collective=True):
    nc = build(collective=_collective)
    in_maps = make_in_maps(np.asarray(x), np.asarray(Wq), np.asarray(Wkv),
                           np.asarray(Wout))
    res = bass_utils.run_bass_kernel_spmd(
        nc, in_maps, core_ids=list(range(N_CORES)), trace=_trace)
    out = np.empty((NTOK, DIM), np.float32)
    for c in range(N_CORES):
        out[c * CH:(c + 1) * CH, :] = res.results[c]["outT"].T
    full = out.reshape(B, SEQ, DIM)
    if _trace:
        return full, res
    return full
